# revision 1
# baseline (speedup 1.0000x reference)
"""GAT-style attention layer (gnn_message_passing) on 8 TRN2 NeuronCores.

Math (reference):
    xf  = X @ W.T                          [N, F1]
    s   = xf @ a0   (att_self,  per-row i)
    t   = xf @ a1   (att_neigh, per-col j)
    att[i,j]   = LeakyReLU_0.2(s_i + t_j)
    E[i,j]     = A[i,j] * exp(att[i,j])      (masked; no max-subtraction
                 needed: |att| < ~25 so exp stays in fp32 range)
    S_j        = sum_i E[i,j]                (softmax axis=0 denominator)
    out[i,g]   = sum_j E[i,j] * xf[j,g] / S_j

Sharding: 1D column (j) shard across 8 cores. Each core owns columns
J_r = [r*1024, (r+1)*1024): it builds E.T[j_local, i] for all i (so the
axis=0 softmax denominator is core-local), aggregates the partial
out[i,:] = sum_{j in J_r} E.T[j,i] * (xf[j,:]/S_j), and one final
ReduceScatter sums partials across cores, handing rank r exactly its
output row block.

The host passes Asc = (A*BIG) as fp16 (exact: A is a 0/1 mask), halving
A's DMA traffic. Per (i-chunk c, j-tile jt) stream unit:
  DMA  : Asc rows (2KB contiguous per partition, fp16)
  DVE  : Am = Asc + (s_i - BIG)   in place (tensor_scalar, 4x mode;
         per-partition s column, so masked entries become ~ -BIG)
  PE   : 8x 128x128 fp16 transposes -> Am.T chunk in PSUM
  DVE  : z = Am.T + t_j           (tensor_scalar from PSUM, 2x mode)
         y = 0.2 * z              (tensor_scalar SBUF, 4x mode)
         w = max(z, y) -> bf16    (tensor_tensor, 2x mode: LeakyReLU)
  ACT  : ET[jt][:, chunk] = Exp(w), accum_out += column sums (fused)
  PE   : aggregation matmuls after all chunks + normalization
All ops avoid scalar_tensor_tensor, which has no fast DVE modes.
(lrelu_k>0 would offload some LeakyReLU tiles to ACT's native Lrelu, but
the HW Lrelu table does not honor alpha=0.2 accurately - measured rel err
2.8e-2 vs 6.7e-3 with the DVE max-form - so the default stays lrelu_k=0.)
"""

import sys

sys.path.insert(0, "/opt/trn_rl_repo")

import numpy as np

import concourse.bass as bass
import concourse.mybir as mybir
from concourse import bacc, tile, masks
from concourse.bass_utils import run_bass_kernel_spmd

N, F, F1 = 8192, 256, 64
NCORES = 8
JL = N // NCORES      # 1024 local columns per core
NT = N // 128         # 64 node tiles (i-tiles)
JT = JL // 128        # 8 local j-tiles per core
FE = F1 + 2           # xf extended with s,t columns
BIG = 30000.0         # additive mask magnitude (fp16-safe)

f32 = mybir.dt.float32
bf16 = mybir.dt.bfloat16
f16 = mybir.dt.float16
Alu = mybir.AluOpType
AF = mybir.ActivationFunctionType


def build_graph(n=N, ncores=NCORES, use_collective=True, reps=1, lrelu_k=0):
    N_, NCORES_ = n, ncores
    JL_ = N_ // NCORES_
    NT_ = N_ // 128
    JT_ = JL_ // 128
    IPC_ = min(8, NT_)          # i-tiles per chunk
    NCH_ = NT_ // IPC_          # chunks
    CW_ = IPC_ * 128            # chunk width in i
    LRELU_K = lrelu_k           # j-tiles whose LeakyReLU runs on ACT
    nc = bacc.Bacc("TRN2", target_bir_lowering=False, num_devices=NCORES_)

    XTl_d = nc.dram_tensor("XTloc", [F, JL_], f32, kind="ExternalInput")
    A_d = nc.dram_tensor("Ash", [N_, JL_], f16, kind="ExternalInput")
    WTe_d = nc.dram_tensor("WTe", [F, FE], f32, kind="ExternalInput")
    out_d = nc.dram_tensor("out", [JL_, F1], f32, kind="ExternalOutput")

    with tile.TileContext(nc) as tc:
        with (
            tc.tile_pool(name="persist", bufs=1) as P,
            tc.tile_pool(name="etp", bufs=1) as ETp,
            tc.tile_pool(name="dram", bufs=1, space="DRAM") as DR,
        ):
            # ---- constants ----
            ident_f16 = P.tile([128, 128], f16)
            masks.make_identity(nc, ident_f16[:])
            ident_f32 = P.tile([128, 128], f32)
            masks.make_identity(nc, ident_f32[:])

            WTe_sb = P.tile([128, 2 * FE], f32)
            nc.sync.dma_start(WTe_sb[:, 0:FE], WTe_d[0:128, :])
            nc.sync.dma_start(WTe_sb[:, FE : 2 * FE], WTe_d[128:256, :])

            # ---- persistent state ----
            ET = [ETp.tile([128, N_], bf16, name=f"et{j}") for j in range(JT_)]
            s_g = P.tile([128, NT_], f32)
            s_g16 = P.tile([128, NT_], f16)
            xf_loc = P.tile([128, JT_ * FE], f32)
            xfn = P.tile([128, JT_ * F1], bf16)
            s_cols = P.tile([128, JT_], f32)
            cs_part = P.tile([128, JT_ * NCH_], f32)
            cs = P.tile([128, JT_], f32)
            rinv = P.tile([128, JT_], f32)

            s_loc_dram = DR.tile([JT_, 128], f16)
            s_all_dram = DR.tile(
                [NT_, 128], f16,
                addr_space="Shared"
                if (NCORES_ > 4 and use_collective)
                else "Local",
            )
            partial_dA = DR.tile([N_ // 2, F1], f32)
            partial_dB = DR.tile([N_ // 2, F1], f32)
            rs_outA = DR.tile([JL_ // 2, F1], f32)
            rs_outB = DR.tile([JL_ // 2, F1], f32)

            for rep_ in range(reps):
                # ================= phase 0: local features + s AllGather ========
                with (
                    tc.tile_pool(name="xstage", bufs=1) as XS,
                    tc.tile_pool(name="xfps", bufs=2, space="PSUM") as XFP,
                    tc.tile_pool(name="scps", bufs=1, space="PSUM") as SCP,
                ):
                    xtl = XS.tile([128, 2 * JL_], f32, name="xtl")
                    nc.sync.dma_start(xtl[:, 0:JL_], XTl_d[0:128, :])
                    nc.sync.dma_start(xtl[:, JL_ : 2 * JL_], XTl_d[128:256, :])
                    for jt in range(JT_):
                        xfp = XFP.tile([128, FE], f32, name="xfp", bufs=2)
                        nc.tensor.matmul(
                            xfp[:],
                            xtl[:, jt * 128 : (jt + 1) * 128],
                            WTe_sb[:, 0:FE],
                            start=True,
                            stop=False,
                        )
                        nc.tensor.matmul(
                            xfp[:],
                            xtl[:, JL_ + jt * 128 : JL_ + (jt + 1) * 128],
                            WTe_sb[:, FE : 2 * FE],
                            start=False,
                            stop=True,
                        )
                        nc.vector.tensor_copy(
                            xf_loc[:, jt * FE : (jt + 1) * FE], xfp[:]
                        )
                        nc.vector.tensor_copy(
                            s_cols[:, jt : jt + 1],
                            xf_loc[:, jt * FE + F1 : jt * FE + F1 + 1],
                        )

                    # local s columns -> rows -> DRAM -> AllGather -> bcast row
                    scp = SCP.tile([JT_, 128], f32, name="scp")
                    nc.tensor.transpose(scp[:], s_cols[:, 0:JT_], ident_f32[:])
                    s_rT = XS.tile([JT_, 128], f16, name="srt", bufs=1)
                    nc.vector.tensor_copy(s_rT[:], scp[:])
                    nc.sync.dma_start(s_loc_dram[:], s_rT[:])
                    if use_collective:
                        nc.gpsimd.collective_compute(
                            "AllGather",
                            Alu.bypass,
                            replica_groups=[list(range(NCORES_))],
                            ins=[s_loc_dram[:].opt()],
                            outs=[s_all_dram[:].opt()],
                        )
                    else:
                        for rr_ in range(NCORES_):
                            nc.sync.dma_start(
                                s_all_dram[rr_ * JT_ : (rr_ + 1) * JT_, :],
                                s_loc_dram[:],
                            )
                    # global s back as per-partition columns [128, NT_]:
                    # transposed read of the [NT_, 128] gather (16KB, strided)
                    nc.sync.dma_start(
                        s_g16[:],
                        s_all_dram[:].rearrange("a b -> b a"),
                    )
                    nc.vector.tensor_copy(s_g[:], s_g16[:])

                # ================= stream: mask+lrelu+exp per (chunk, j-tile) ===
                with (
                    tc.tile_pool(name="amsk", bufs=IPC_ + 4) as ABP,
                    tc.tile_pool(name="tpps", bufs=4, space="PSUM") as TPP,
                    tc.tile_pool(name="upool", bufs=2) as UPP,
                    tc.tile_pool(name="zpool", bufs=4) as ZP,
                ):
                    for c in range(NCH_):
                        am_tiles = []
                        for q in range(IPC_):
                            tau = c * IPC_ + q
                            am = ABP.tile([128, JL_], f16, name="am")
                            nc.sync.dma_start(
                                am[:], A_d[tau * 128 : (tau + 1) * 128, :]
                            )
                            # Am = Asc + (s_i - BIG), in place (4x single-src)
                            nc.vector.tensor_scalar(
                                am[:], am[:], s_g[:, tau : tau + 1], -BIG,
                                Alu.add, Alu.add,
                            )
                            am_tiles.append(am)
                        for jt in range(JT_):
                            tp = TPP.tile([128, CW_], f16, name="tp")
                            for q in range(IPC_):
                                nc.tensor.transpose(
                                    tp[:, q * 128 : (q + 1) * 128],
                                    am_tiles[q][:, jt * 128 : (jt + 1) * 128],
                                    ident_f16[:],
                                )
                            t_ap = xf_loc[:, jt * FE + F1 + 1 : jt * FE + F1 + 2]
                            if jt < LRELU_K:
                                # ACT-path LeakyReLU: balances DVE load
                                u = UPP.tile([128, CW_], f16, name="u")
                                nc.scalar.activation(
                                    u[:], tp[:], AF.Lrelu,
                                    bias=t_ap, scale=1.0, alpha=0.2,
                                )
                                nc.scalar.activation(
                                    ET[jt][:, c * CW_ : (c + 1) * CW_],
                                    u[:],
                                    AF.Exp,
                                    accum_out=cs_part[:, jt * NCH_ + c : jt * NCH_ + c + 1],
                                )
                            else:
                                z = ZP.tile([128, CW_], f16, name="z")
                                nc.vector.tensor_scalar(
                                    z[:], tp[:], t_ap, None, Alu.add
                                )
                                y = ZP.tile([128, CW_], f16, name="y")
                                nc.vector.tensor_scalar(
                                    y[:], z[:], 0.2, None, Alu.mult
                                )
                                w = ZP.tile([128, CW_], bf16, name="w")
                                nc.vector.tensor_tensor(
                                    w[:], z[:], y[:], Alu.max
                                )
                                nc.scalar.activation(
                                    ET[jt][:, c * CW_ : (c + 1) * CW_],
                                    w[:],
                                    AF.Exp,
                                    accum_out=cs_part[:, jt * NCH_ + c : jt * NCH_ + c + 1],
                                )

                # ================= tail: normalize, aggregate, reduce ============
                with (
                    tc.tile_pool(name="aggps", bufs=6, space="PSUM") as AGP,
                    tc.tile_pool(name="ocp", bufs=1) as OCP,
                ):
                    for jt in range(JT_):
                        nc.vector.tensor_reduce(
                            cs[:, jt : jt + 1],
                            cs_part[:, jt * NCH_ : (jt + 1) * NCH_],
                            axis=mybir.AxisListType.X,
                            op=Alu.add,
                        )
                    nc.vector.reciprocal(rinv[:], cs[:])
                    for jt in range(JT_):
                        nc.vector.tensor_scalar(
                            xfn[:, jt * F1 : (jt + 1) * F1],
                            xf_loc[:, jt * FE : jt * FE + F1],
                            rinv[:, jt : jt + 1],
                            None,
                            Alu.mult,
                        )
                    # Aggregate in two halves: half H holds row-blocks b
                    # with (b mod 8) < 4 (H=0) or >= 4 (H=1), packed so the
                    # ReduceScatter of half H hands rank r exactly rows
                    # [r*JL + H*JL/2, r*JL + (H+1)*JL/2).
                    halves = [
                        (partial_dA, rs_outA, 0),
                        (partial_dB, rs_outB, 1),
                    ]
                    hb = JT_ // 2  # row-blocks per rank per half
                    for part_d, rs_o, H in halves:
                        stage = OCP.tile(
                            [128, NT_ // 2 * F1], f32, name=f"stage{H}"
                        )
                        for rb in range(NT_ // 2):
                            b = (rb // hb) * JT_ + (rb % hb) + H * hb
                            ag = AGP.tile([128, F1], f32, name="ag")
                            for jt in range(JT_):
                                nc.tensor.matmul(
                                    ag[:],
                                    ET[jt][:, b * 128 : (b + 1) * 128],
                                    xfn[:, jt * F1 : (jt + 1) * F1],
                                    start=(jt == 0),
                                    stop=(jt == JT_ - 1),
                                )
                            if rb % 2 == 0:
                                nc.scalar.copy(
                                    stage[:, rb * F1 : (rb + 1) * F1], ag[:]
                                )
                            else:
                                nc.vector.tensor_copy(
                                    stage[:, rb * F1 : (rb + 1) * F1], ag[:]
                                )
                        nc.sync.dma_start(
                            part_d[:].rearrange("(b p) g -> p b g", p=128),
                            stage[:].rearrange("p (b g) -> p b g", g=F1),
                        )
                        if use_collective:
                            nc.gpsimd.collective_compute(
                                "ReduceScatter",
                                Alu.add,
                                replica_groups=[list(range(NCORES_))],
                                ins=[part_d[:].opt()],
                                outs=[rs_o[:].opt()],
                            )
                            nc.sync.dma_start(
                                out_d[
                                    H * (JL_ // 2) : (H + 1) * (JL_ // 2), :
                                ],
                                rs_o[:],
                            )
                        else:
                            nc.sync.dma_start(
                                out_d[
                                    H * (JL_ // 2) : (H + 1) * (JL_ // 2), :
                                ],
                                part_d[0 : JL_ // 2, :],
                            )

    nc.compile()
    return nc


_GRAPH = None


def make_in_maps(X, A, W, a):
    X = np.asarray(X, dtype=np.float32)
    A = np.asarray(A, dtype=np.float32)
    W = np.asarray(W, dtype=np.float32)
    a = np.asarray(a, dtype=np.float32)

    WT = W.T.astype(np.float32)                      # [256, 64]
    WTe = np.concatenate([WT, WT @ a[0], WT @ a[1]], axis=1)  # [256, 66]
    WTe = np.ascontiguousarray(WTe, dtype=np.float32)

    in_maps = []
    for r in range(NCORES):
        in_maps.append(
            {
                "XTloc": np.ascontiguousarray(X[r * JL : (r + 1) * JL].T),
                "Ash": np.ascontiguousarray(
                    (A[:, r * JL : (r + 1) * JL] * BIG).astype(np.float16)
                ),
                "WTe": WTe,
            }
        )
    return in_maps


def kernel(X, A, W, a):
    global _GRAPH
    if _GRAPH is None:
        _GRAPH = build_graph()
    nc = _GRAPH

    in_maps = make_in_maps(X, A, W, a)
    res = run_bass_kernel_spmd(nc, in_maps, list(range(NCORES)))
    out = np.concatenate(
        [res.results[r]["out"] for r in range(NCORES)], axis=0
    )
    return out.astype(np.float32)



# revision 4
# speedup vs baseline: 1.0076x; 1.0076x over previous
"""GAT-style attention layer (gnn_message_passing) on 8 TRN2 NeuronCores.

Math (reference):
    xf  = X @ W.T                          [N, F1]
    s   = xf @ a0   (att_self,  per-row i)
    t   = xf @ a1   (att_neigh, per-col j)
    att[i,j]   = LeakyReLU_0.2(s_i + t_j)
    E[i,j]     = A[i,j] * exp(att[i,j])
    S_j        = sum_i E[i,j]              (softmax axis=0 denominator)
    out[i,g]   = sum_j E[i,j] * xf[j,g] / S_j

Sharding: 1D column (j) shard across 8 cores; core r owns j in
[r*1024, (r+1)*1024). The axis=0 softmax denominator is core-local.

v2 design (vs. the transpose-on-device baseline):
  * The host passes AshT = ((A[:, loc].T) - 1) * BIG as fp16
    [JL, N] — already TRANSPOSED (j on rows) and affine-folded so
    edges are 0 and non-edges -BIG.  The device does NO PE transposes
    and no PSUM round-trip for the big tensor.
  * s_i must be added along the FREE axis, so the kernel builds
    s_bcast [128, N] (s broadcast to every partition) with PE matmuls:
    stationary = (W.T@a0) replicated across 128 columns (host input
    WsB), moving = replicated fp16 X.T — out[p, i] = s_i for every p.
    This replaces the baseline's AllGather of s entirely.
  * One custom DVE op (registered into concourse.dve_ops at import)
    computes   w = max(z, 0.2*z),  z = (at + t_j) + s_i
    in a SINGLE 1x pass (4 ALU slices): in0 = AshT tile (in-place),
    in1 = s_bcast, s0 = t column (per-partition), imm2 = 0.2.
    The baseline needed 3-4 DVE instructions for the same elements.
  * ACT does E = Exp(w) with accum_out giving the per-j row sums
    (softmax denominators) for free.
  * Aggregation runs PER-jt into a persistent PSUM accumulator
    [128, 64*64] (all 8 banks): after jt's exp, xfn = xf*1/S, then 64
    matmuls ag[b] += ET.T chunk @ xfn, start at jt==0, stop at jt==7.
    This hides the aggregation under the stream; the tail is only
    PSUM->SBUF copies, one 2MB DMA, one ReduceScatter.

Per-core engine budget (errata-adjusted cost model):
  DVE  8 x (58+8192)      = 66k cyc @0.96GHz = 69us   <- bottleneck
  ACT  8 x (352+8192)     = 68k cyc @1.2GHz  = 57us
  DMA  16MB A + 5MB X/misc                   = 59us
  PE   512 agg matmuls + prefix              = 25us
"""

import sys

sys.path.insert(0, "/opt/trn_rl_repo")

import numpy as np

import concourse.bass as bass
import concourse.mybir as mybir
from concourse import bacc, tile
from concourse.bass_utils import run_bass_kernel_spmd

N, F, F1 = 8192, 256, 64
NCORES = 8
JL = N // NCORES      # 1024 local columns per core
NT = N // 128         # 64 node tiles (i-tiles)
JT = JL // 128        # 8 local j-tiles per core
FE = F1 + 2           # xf extended with s,t columns
BIG = 30000.0         # additive mask magnitude (fp16-exact)

f32 = mybir.dt.float32
bf16 = mybir.dt.bfloat16
f16 = mybir.dt.float16
Alu = mybir.AluOpType
AF = mybir.ActivationFunctionType


# --------------------------------------------------------------------------
# Custom DVE op: w = max(z, imm2*z), z = (in0 + s0) + in1.  One 1x pass
# replaces the baseline's tensor_scalar + tensor_tensor + tensor_scalar +
# tensor_tensor chain.  Registered into concourse.dve_ops' name->row map at
# import (the documented extension point; row 5-bit field has free slots).
# --------------------------------------------------------------------------
_LRELU_OP = None


def _register_lrelu_op():
    global _LRELU_OP
    if _LRELU_OP is not None:
        return _LRELU_OP
    import concourse.dve_ops as DOPS
    from concourse.dve_spec import C0, C2, Spec, Src0, Src1, lower, maxx
    from concourse.dve_uop import DveOpSpec

    name = "LRELU_ADD2_ANT"
    if name in DOPS.CUSTOM_DVE_SPECS:
        _LRELU_OP = next(op for op in DOPS.OPS if op.name == name)
        return _LRELU_OP

    z = (Src0 + C0) + Src1

    def _ref(in0, in1, s0, s1, imm2):
        zf = in0.astype(np.float32) + s0 + in1.astype(np.float32)
        return np.maximum(zf, zf * imm2)

    spec = Spec(body=maxx(z, z * C2), reference=_ref)

    row = DOPS._CUSTOM_DVE_ROW_BASE + len(DOPS.OPS)
    assert row < 0x20
    DOPS._SUB_OPCODE_FOR_NAME[name] = row
    shas = {}
    for ver in ("v3", "v4"):
        uops = lower(spec, ver=ver)
        shas[ver] = DveOpSpec(
            name=name, opcode=row, uops=uops, rd1_en=True
        ).sha(ver)
    op = DOPS.DveOp(name, spec, subdim=False, uops_sha=shas)
    DOPS.OPS.append(op)
    DOPS.CUSTOM_DVE_SPECS[name] = spec
    _LRELU_OP = op
    return op


def build_graph(n=N, ncores=NCORES, use_collective=True, reps=1):
    N_, NCORES_ = n, ncores
    JL_ = N_ // NCORES_
    NT_ = N_ // 128
    JT_ = JL_ // 128
    SCH = 2048                  # s_bcast build chunk (columns)
    NSCH = N_ // SCH
    lrelu = _register_lrelu_op()
    nc = bacc.Bacc("TRN2", target_bir_lowering=False, num_devices=NCORES_)

    AshT_d = nc.dram_tensor("AshT", [JL_, N_], f16, kind="ExternalInput")
    XTl_d = nc.dram_tensor("XTloc", [F, JL_], f32, kind="ExternalInput")
    XTf_d = nc.dram_tensor("XTfull", [F, N_], f16, kind="ExternalInput")
    WTe_d = nc.dram_tensor("WTe", [F, FE], f32, kind="ExternalInput")
    WsB_d = nc.dram_tensor("WsB", [F, 128], f16, kind="ExternalInput")
    out_d = nc.dram_tensor("out", [JL_, F1], f32, kind="ExternalOutput")

    with tile.TileContext(nc) as tc:
        with (
            tc.tile_pool(name="persist", bufs=1) as P,
            tc.tile_pool(name="dram", bufs=1, space="DRAM") as DR,
        ):
            WTe_sb = P.tile([128, 2 * FE], f32)
            nc.sync.dma_start(WTe_sb[:, 0:FE], WTe_d[0:128, :])
            nc.sync.dma_start(WTe_sb[:, FE : 2 * FE], WTe_d[128:256, :])
            wsb_sb = P.tile([128, 2 * 128], f16)
            nc.sync.dma_start(wsb_sb[:, 0:128], WsB_d[0:128, :])
            nc.sync.dma_start(wsb_sb[:, 128:256], WsB_d[128:256, :])

            s_bcast = P.tile([128, N_], f16)
            xf_loc = P.tile([128, JT_ * FE], f32)
            cs = P.tile([128, JT_], f32)
            rinv = P.tile([128, JT_], f32)
            xfn = P.tile([128, JT_ * F1], bf16)

            partial_d = DR.tile([N_, F1], f32)
            rs_out = DR.tile([JL_, F1], f32)

            for rep_ in range(reps):
                # ===== phase 0: local features xf + s broadcast row ======
                with (
                    tc.tile_pool(name="xstage", bufs=1) as XS,
                    tc.tile_pool(name="xfps", bufs=2, space="PSUM") as XFP,
                    tc.tile_pool(name="scps", bufs=4, space="PSUM") as SCP,
                ):
                    xtl = XS.tile([128, 2 * JL_], f32, name="xtl")
                    nc.sync.dma_start(xtl[:, 0:JL_], XTl_d[0:128, :])
                    nc.sync.dma_start(xtl[:, JL_ : 2 * JL_], XTl_d[128:256, :])
                    for jt in range(JT_):
                        xfp = XFP.tile([128, FE], f32, name="xfp", bufs=2)
                        nc.tensor.matmul(
                            xfp[:],
                            xtl[:, jt * 128 : (jt + 1) * 128],
                            WTe_sb[:, 0:FE],
                            start=True,
                            stop=False,
                        )
                        nc.tensor.matmul(
                            xfp[:],
                            xtl[:, JL_ + jt * 128 : JL_ + (jt + 1) * 128],
                            WTe_sb[:, FE : 2 * FE],
                            start=False,
                            stop=True,
                        )
                        nc.vector.tensor_copy(
                            xf_loc[:, jt * FE : (jt + 1) * FE], xfp[:]
                        )

                    # s_bcast[p, i] = s_i for every p: stationary = ws
                    # replicated over 128 cols, moving = full fp16 X.T.
                    # Chunked so the stream's first DVE op can start as
                    # soon as the first chunks land.
                    xtf = XS.tile([128, 2 * N_], f16, name="xtf")
                    for c in range(NSCH):
                        for h in (0, 1):
                            nc.sync.dma_start(
                                xtf[:, h * N_ + c * SCH : h * N_ + (c + 1) * SCH],
                                XTf_d[h * 128 : (h + 1) * 128, c * SCH : (c + 1) * SCH],
                            )
                        for q in range(SCH // 512):
                            col = c * SCH + q * 512
                            scp = SCP.tile([128, 512], f32, name="scp", bufs=4)
                            nc.tensor.matmul(
                                scp[:],
                                wsb_sb[:, 0:128],
                                xtf[:, col : col + 512],
                                start=True,
                                stop=False,
                            )
                            nc.tensor.matmul(
                                scp[:],
                                wsb_sb[:, 128:256],
                                xtf[:, N_ + col : N_ + col + 512],
                                start=False,
                                stop=True,
                            )
                            if q % 2 == 0:
                                nc.scalar.copy(
                                    s_bcast[:, col : col + 512], scp[:]
                                )
                            else:
                                nc.vector.tensor_copy(
                                    s_bcast[:, col : col + 512], scp[:]
                                )

                # ===== stream: one fused DVE op + one exp per j-tile =====
                with (
                    tc.tile_pool(name="atp", bufs=3) as ATP,
                    tc.tile_pool(name="etp", bufs=2) as ETP,
                    tc.tile_pool(name="aggps", bufs=1, space="PSUM") as AGP,
                    tc.tile_pool(name="ocp", bufs=1) as OCP,
                ):
                    ag = AGP.tile([128, NT_ * F1], f32, name="ag")
                    for jt in range(JT_):
                        at = ATP.tile([128, N_], f16, name="at")
                        nc.sync.dma_start(
                            at[:], AshT_d[jt * 128 : (jt + 1) * 128, :]
                        )
                        t_ap = xf_loc[:, jt * FE + F1 + 1 : jt * FE + F1 + 2]
                        # w = max(z, 0.2z), z = at + t_j + s_i, in place.
                        # jt==0 is chunked to overlap with the s_bcast build.
                        nch = NSCH if jt == 0 else 1
                        cw = N_ // nch
                        for c in range(nch):
                            nc.vector._custom_dve(
                                lrelu,
                                out=at[:, c * cw : (c + 1) * cw],
                                in0=at[:, c * cw : (c + 1) * cw],
                                in1=s_bcast[:, c * cw : (c + 1) * cw],
                                s0=t_ap,
                                imm2=0.2,
                            )
                        et = ETP.tile([128, N_], bf16, name="et")
                        nc.scalar.activation(
                            et[:],
                            at[:],
                            AF.Exp,
                            accum_out=cs[:, jt : jt + 1],
                        )
                        nc.vector.reciprocal(
                            rinv[:, jt : jt + 1], cs[:, jt : jt + 1]
                        )
                        nc.vector.tensor_scalar(
                            xfn[:, jt * F1 : (jt + 1) * F1],
                            xf_loc[:, jt * FE : jt * FE + F1],
                            rinv[:, jt : jt + 1],
                            None,
                            Alu.mult,
                        )
                        for b in range(NT_):
                            # start=True clears has_written for the WHOLE
                            # bank, so issue it only on the first region of
                            # each bank; later regions' first writes
                            # overwrite-then-accumulate via has_written.
                            nc.tensor.matmul(
                                ag[:, b * F1 : (b + 1) * F1],
                                et[:, b * 128 : (b + 1) * 128],
                                xfn[:, jt * F1 : (jt + 1) * F1],
                                start=(jt == 0 and b % 8 == 0),
                                stop=(jt == JT_ - 1),
                            )

                    # ===== tail: PSUM -> SBUF -> DRAM -> ReduceScatter ====
                    stage = OCP.tile([128, NT_ * F1], f32, name="stage")
                    for k in range(8):
                        sl = slice(k * 512, (k + 1) * 512)
                        if k % 2 == 0:
                            nc.scalar.copy(stage[:, sl], ag[:, sl])
                        else:
                            nc.vector.tensor_copy(stage[:, sl], ag[:, sl])
                    nc.sync.dma_start(
                        partial_d[:].rearrange("(b p) g -> p b g", p=128),
                        stage[:].rearrange("p (b g) -> p b g", g=F1),
                    )
                    if use_collective:
                        nc.gpsimd.collective_compute(
                            "ReduceScatter",
                            Alu.add,
                            replica_groups=[list(range(NCORES_))],
                            ins=[partial_d[:].opt()],
                            outs=[rs_out[:].opt()],
                        )
                        nc.sync.dma_start(out_d[:], rs_out[:])
                    else:
                        nc.sync.dma_start(out_d[:], partial_d[0:JL_, :])

    nc.compile()
    return nc


_GRAPH = None


def make_in_maps(X, A, W, a):
    X = np.asarray(X, dtype=np.float32)
    A = np.asarray(A, dtype=np.float32)
    W = np.asarray(W, dtype=np.float32)
    a = np.asarray(a, dtype=np.float32)

    WT = W.T.astype(np.float32)                               # [256, 64]
    WTe = np.concatenate([WT, WT @ a[0], WT @ a[1]], axis=1)  # [256, 66]
    WTe = np.ascontiguousarray(WTe, dtype=np.float32)
    WsB = np.ascontiguousarray(
        np.repeat(WT @ a[0], 128, axis=1), dtype=np.float16
    )                                                          # [256, 128]
    XTf = np.ascontiguousarray(X.T).astype(np.float16)         # [256, 8192]

    in_maps = []
    for r in range(NCORES):
        sl = slice(r * JL, (r + 1) * JL)
        in_maps.append(
            {
                "AshT": np.ascontiguousarray(
                    (A[:, sl].T - 1.0) * BIG
                ).astype(np.float16),
                "XTloc": np.ascontiguousarray(X[sl].T),
                "XTfull": XTf,
                "WTe": WTe,
                "WsB": WsB,
            }
        )
    return in_maps


def kernel(X, A, W, a):
    global _GRAPH
    if _GRAPH is None:
        _GRAPH = build_graph()
    nc = _GRAPH

    in_maps = make_in_maps(X, A, W, a)
    res = run_bass_kernel_spmd(nc, in_maps, list(range(NCORES)))
    out = np.concatenate(
        [res.results[r]["out"] for r in range(NCORES)], axis=0
    )
    return out.astype(np.float32)


# revision 8
# speedup vs baseline: 1.1559x; 1.1472x over previous
"""GAT-style attention layer (gnn_message_passing) on 8 TRN2 NeuronCores.

Math (reference):
    xf  = X @ W.T                          [N, F1]
    s   = xf @ a0   (att_self,  per-row i)
    t   = xf @ a1   (att_neigh, per-col j)
    att[i,j]   = LeakyReLU_0.2(s_i + t_j)
    E[i,j]     = A[i,j] * exp(att[i,j])
    S_j        = sum_i E[i,j]              (softmax axis=0 denominator)
    out[i,g]   = sum_j E[i,j] * xf[j,g] / S_j

Sharding: 1D column (j) shard across 8 cores; core r owns j in
[r*1024, (r+1)*1024). The axis=0 softmax denominator is core-local.

v2 design (vs. the transpose-on-device baseline):
  * The host passes AshT = ((A[:, loc].T) - 1) * BIG as fp16
    [JL, N] — already TRANSPOSED (j on rows) and affine-folded so
    edges are 0 and non-edges -BIG.  The device does NO PE transposes
    and no PSUM round-trip for the big tensor.
  * s_i must be added along the FREE axis, so the kernel builds
    s_bcast [128, N] (s broadcast to every partition) with PE matmuls:
    stationary = (W.T@a0) replicated across 128 columns (host input
    WsB), moving = replicated fp16 X.T — out[p, i] = s_i for every p.
    This replaces the baseline's AllGather of s entirely.
  * One custom DVE op (registered into concourse.dve_ops at import)
    computes   w = max(z, 0.2*z),  z = (at + t_j) + s_i
    in a SINGLE 1x pass (4 ALU slices): in0 = AshT tile (in-place),
    in1 = s_bcast, s0 = t column (per-partition), imm2 = 0.2.
    The baseline needed 3-4 DVE instructions for the same elements.
  * ACT does E = Exp(w) with accum_out giving the per-j row sums
    (softmax denominators) for free.
  * Aggregation runs PER-jt into a persistent PSUM accumulator
    [128, 64*64] (all 8 banks): after jt's exp, xfn = xf*1/S, then 64
    matmuls ag[b] += ET.T chunk @ xfn, start at jt==0, stop at jt==7.
    This hides the aggregation under the stream; the tail is only
    PSUM->SBUF copies, one 2MB DMA, one ReduceScatter.

Per-core engine budget (errata-adjusted cost model):
  DVE  8 x (58+8192)      = 66k cyc @0.96GHz = 69us   <- bottleneck
  ACT  8 x (352+8192)     = 68k cyc @1.2GHz  = 57us
  DMA  16MB A + 5MB X/misc                   = 59us
  PE   512 agg matmuls + prefix              = 25us
"""

import sys

sys.path.insert(0, "/opt/trn_rl_repo")

import numpy as np

import concourse.bass as bass
import concourse.mybir as mybir
from concourse import bacc, tile
from concourse.bass_utils import run_bass_kernel_spmd

N, F, F1 = 8192, 256, 64
NCORES = 8
JL = N // NCORES      # 1024 local columns per core
NT = N // 128         # 64 node tiles (i-tiles)
JT = JL // 128        # 8 local j-tiles per core
FE = F1 + 2           # xf extended with s,t columns
BIG = 30000.0         # additive mask magnitude (fp16-exact)

f32 = mybir.dt.float32
bf16 = mybir.dt.bfloat16
f16 = mybir.dt.float16
Alu = mybir.AluOpType
AF = mybir.ActivationFunctionType


# --------------------------------------------------------------------------
# Custom DVE op: w = max(z, imm2*z), z = (in0 + s0) + in1.  One 1x pass
# replaces the baseline's tensor_scalar + tensor_tensor + tensor_scalar +
# tensor_tensor chain.  Registered into concourse.dve_ops' name->row map at
# import (the documented extension point; row 5-bit field has free slots).
# --------------------------------------------------------------------------
_LRELU_OP = None


def _register_lrelu_op():
    global _LRELU_OP
    if _LRELU_OP is not None:
        return _LRELU_OP
    import concourse.dve_ops as DOPS
    from concourse.dve_spec import C0, C2, Spec, Src0, Src1, lower, maxx
    from concourse.dve_uop import DveOpSpec

    name = "LRELU_ADD2_ANT"
    if name in DOPS.CUSTOM_DVE_SPECS:
        _LRELU_OP = next(op for op in DOPS.OPS if op.name == name)
        return _LRELU_OP

    z = (Src0 + C0) + Src1

    def _ref(in0, in1, s0, s1, imm2):
        zf = in0.astype(np.float32) + s0 + in1.astype(np.float32)
        return np.maximum(zf, zf * imm2)

    spec = Spec(body=maxx(z, z * C2), reference=_ref)

    row = DOPS._CUSTOM_DVE_ROW_BASE + len(DOPS.OPS)
    assert row < 0x20
    DOPS._SUB_OPCODE_FOR_NAME[name] = row
    shas = {}
    for ver in ("v3", "v4"):
        uops = lower(spec, ver=ver)
        shas[ver] = DveOpSpec(
            name=name, opcode=row, uops=uops, rd1_en=True
        ).sha(ver)
    op = DOPS.DveOp(name, spec, subdim=False, uops_sha=shas)
    DOPS.OPS.append(op)
    DOPS.CUSTOM_DVE_SPECS[name] = spec
    _LRELU_OP = op
    return op


def build_graph(n=N, ncores=NCORES, use_collective=True, reps=1):
    N_, NCORES_ = n, ncores
    JL_ = N_ // NCORES_
    NT_ = N_ // 128
    JT_ = JL_ // 128
    SCH = 2048                  # s_bcast build chunk (columns)
    NSCH = N_ // SCH
    lrelu = _register_lrelu_op()
    nc = bacc.Bacc("TRN2", target_bir_lowering=False, num_devices=NCORES_)

    AshT_d = nc.dram_tensor("AshT", [JL_, N_], f16, kind="ExternalInput")
    XTl_d = nc.dram_tensor("XTloc", [F, JL_], f32, kind="ExternalInput")
    XTf_d = nc.dram_tensor("XTfull", [F, N_], f16, kind="ExternalInput")
    WTe_d = nc.dram_tensor("WTe", [F, FE], f32, kind="ExternalInput")
    WsB_d = nc.dram_tensor("WsB", [F, 128], f16, kind="ExternalInput")
    # fp16 output + fp16 RS partials: halves the ReduceScatter bytes (the
    # single biggest exposed cost) and the output DMA; host upcasts.
    out_d = nc.dram_tensor("out", [JL_, F1], f16, kind="ExternalOutput")

    with tile.TileContext(nc) as tc:
        with (
            tc.tile_pool(name="persist", bufs=1) as P,
            tc.tile_pool(name="dram", bufs=1, space="DRAM") as DR,
        ):
            WTe_sb = P.tile([128, 2 * FE], f32)
            nc.sync.dma_start(WTe_sb[:, 0:FE], WTe_d[0:128, :])
            nc.sync.dma_start(WTe_sb[:, FE : 2 * FE], WTe_d[128:256, :])
            wsb_sb = P.tile([128, 2 * 128], f16)
            nc.sync.dma_start(wsb_sb[:, 0:128], WsB_d[0:128, :])
            nc.sync.dma_start(wsb_sb[:, 128:256], WsB_d[128:256, :])

            s_bcast = P.tile([128, N_], f16)
            xf_loc = P.tile([128, JT_ * FE], f32)
            cs = P.tile([128, JT_], f32)
            rinv = P.tile([128, JT_], f32)
            xfn = P.tile([128, JT_ * F1], bf16)

            partial_d = DR.tile([N_, F1], f16)
            rs_out = DR.tile([JL_, F1], f16)

            for rep_ in range(reps):
                # ===== phase 0: local features xf + s broadcast row ======
                with (
                    tc.tile_pool(name="xstage", bufs=1) as XS,
                    tc.tile_pool(name="xfps", bufs=2, space="PSUM") as XFP,
                    tc.tile_pool(name="scps", bufs=4, space="PSUM") as SCP,
                ):
                    xtl = XS.tile([128, 2 * JL_], f32, name="xtl")
                    nc.sync.dma_start(xtl[:, 0:JL_], XTl_d[0:128, :])
                    nc.sync.dma_start(xtl[:, JL_ : 2 * JL_], XTl_d[128:256, :])
                    for jt in range(JT_):
                        xfp = XFP.tile([128, FE], f32, name="xfp", bufs=2)
                        nc.tensor.matmul(
                            xfp[:],
                            xtl[:, jt * 128 : (jt + 1) * 128],
                            WTe_sb[:, 0:FE],
                            start=True,
                            stop=False,
                        )
                        nc.tensor.matmul(
                            xfp[:],
                            xtl[:, JL_ + jt * 128 : JL_ + (jt + 1) * 128],
                            WTe_sb[:, FE : 2 * FE],
                            start=False,
                            stop=True,
                        )
                        nc.vector.tensor_copy(
                            xf_loc[:, jt * FE : (jt + 1) * FE], xfp[:]
                        )

                    # s_bcast[p, i] = s_i for every p: stationary = ws
                    # replicated over 128 cols, moving = full fp16 X.T.
                    # Chunked so the stream's first DVE op can start as
                    # soon as the first chunks land.
                    xtf = XS.tile([128, 2 * N_], f16, name="xtf")
                    for c in range(NSCH):
                        for h in (0, 1):
                            nc.sync.dma_start(
                                xtf[:, h * N_ + c * SCH : h * N_ + (c + 1) * SCH],
                                XTf_d[h * 128 : (h + 1) * 128, c * SCH : (c + 1) * SCH],
                            )
                        for q in range(SCH // 512):
                            col = c * SCH + q * 512
                            scp = SCP.tile([128, 512], f32, name="scp", bufs=4)
                            nc.tensor.matmul(
                                scp[:],
                                wsb_sb[:, 0:128],
                                xtf[:, col : col + 512],
                                start=True,
                                stop=False,
                            )
                            nc.tensor.matmul(
                                scp[:],
                                wsb_sb[:, 128:256],
                                xtf[:, N_ + col : N_ + col + 512],
                                start=False,
                                stop=True,
                            )
                            if q % 2 == 0:
                                nc.scalar.copy(
                                    s_bcast[:, col : col + 512], scp[:]
                                )
                            else:
                                nc.vector.tensor_copy(
                                    s_bcast[:, col : col + 512], scp[:]
                                )

                # ===== stream: one fused DVE op + one exp per j-tile =====
                with (
                    tc.tile_pool(name="atp", bufs=3) as ATP,
                    tc.tile_pool(name="etp", bufs=2) as ETP,
                    tc.tile_pool(name="aggps", bufs=1, space="PSUM") as AGP,
                    tc.tile_pool(name="ocp", bufs=1) as OCP,
                ):
                    ag = AGP.tile([128, NT_ * F1], f32, name="ag")
                    for jt in range(JT_):
                        at = ATP.tile([128, N_], f16, name="at")
                        nc.sync.dma_start(
                            at[:], AshT_d[jt * 128 : (jt + 1) * 128, :]
                        )
                        t_ap = xf_loc[:, jt * FE + F1 + 1 : jt * FE + F1 + 2]
                        # w = max(z, 0.2z), z = at + t_j + s_i, in place.
                        # jt==0 is chunked to overlap with the s_bcast build.
                        nch = NSCH if jt == 0 else 1
                        cw = N_ // nch
                        for c in range(nch):
                            nc.vector._custom_dve(
                                lrelu,
                                out=at[:, c * cw : (c + 1) * cw],
                                in0=at[:, c * cw : (c + 1) * cw],
                                in1=s_bcast[:, c * cw : (c + 1) * cw],
                                s0=t_ap,
                                imm2=0.2,
                            )
                        et = ETP.tile([128, N_], bf16, name="et")
                        nc.scalar.activation(
                            et[:],
                            at[:],
                            AF.Exp,
                            accum_out=cs[:, jt : jt + 1],
                        )
                        nc.vector.reciprocal(
                            rinv[:, jt : jt + 1], cs[:, jt : jt + 1]
                        )
                        nc.vector.tensor_scalar(
                            xfn[:, jt * F1 : (jt + 1) * F1],
                            xf_loc[:, jt * FE : jt * FE + F1],
                            rinv[:, jt : jt + 1],
                            None,
                            Alu.mult,
                        )
                        for b in range(NT_):
                            # start=True clears has_written for the WHOLE
                            # bank, so issue it only on the first region of
                            # each bank; later regions' first writes
                            # overwrite-then-accumulate via has_written.
                            nc.tensor.matmul(
                                ag[:, b * F1 : (b + 1) * F1],
                                et[:, b * 128 : (b + 1) * 128],
                                xfn[:, jt * F1 : (jt + 1) * F1],
                                start=(jt == 0 and b % 8 == 0),
                                stop=(jt == JT_ - 1),
                            )

                    # ===== tail: PSUM -> SBUF -> DRAM -> ReduceScatter ====
                    stage = OCP.tile([128, NT_ * F1], f16, name="stage")
                    for k in range(8):
                        sl = slice(k * 512, (k + 1) * 512)
                        if k % 2 == 0:
                            nc.scalar.copy(stage[:, sl], ag[:, sl])
                        else:
                            nc.vector.tensor_copy(stage[:, sl], ag[:, sl])
                    nc.sync.dma_start(
                        partial_d[:].rearrange("(b p) g -> p b g", p=128),
                        stage[:].rearrange("p (b g) -> p b g", g=F1),
                    )
                    if use_collective:
                        nc.gpsimd.collective_compute(
                            "ReduceScatter",
                            Alu.add,
                            replica_groups=[list(range(NCORES_))],
                            ins=[partial_d[:].opt()],
                            outs=[rs_out[:].opt()],
                        )
                        nc.sync.dma_start(out_d[:], rs_out[:])
                    else:
                        nc.sync.dma_start(out_d[:], partial_d[0:JL_, :])

    nc.compile()
    return nc


_GRAPH = None


def make_in_maps(X, A, W, a):
    X = np.asarray(X, dtype=np.float32)
    A = np.asarray(A, dtype=np.float32)
    W = np.asarray(W, dtype=np.float32)
    a = np.asarray(a, dtype=np.float32)

    WT = W.T.astype(np.float32)                               # [256, 64]
    WTe = np.concatenate([WT, WT @ a[0], WT @ a[1]], axis=1)  # [256, 66]
    WTe = np.ascontiguousarray(WTe, dtype=np.float32)
    WsB = np.ascontiguousarray(
        np.repeat(WT @ a[0], 128, axis=1), dtype=np.float16
    )                                                          # [256, 128]
    XTf = np.ascontiguousarray(X.T).astype(np.float16)         # [256, 8192]

    in_maps = []
    for r in range(NCORES):
        sl = slice(r * JL, (r + 1) * JL)
        in_maps.append(
            {
                "AshT": np.ascontiguousarray(
                    (A[:, sl].T - 1.0) * BIG
                ).astype(np.float16),
                "XTloc": np.ascontiguousarray(X[sl].T),
                "XTfull": XTf,
                "WTe": WTe,
                "WsB": WsB,
            }
        )
    return in_maps


def kernel(X, A, W, a):
    global _GRAPH
    if _GRAPH is None:
        _GRAPH = build_graph()
    nc = _GRAPH

    in_maps = make_in_maps(X, A, W, a)
    res = run_bass_kernel_spmd(nc, in_maps, list(range(NCORES)))
    out = np.concatenate(
        [res.results[r]["out"] for r in range(NCORES)], axis=0
    )
    return out.astype(np.float32)
